# revision 25
# baseline (speedup 1.0000x reference)
"""Trainium2 Bass kernel for nn_FusedKQnA (sparse attention with learned
queries + depthwise stride-2 conv aggregation).

Math restructuring (vs the reference):
  - k is never materialized: qkT = x^T @ (Wk @ QW) with QW the block-diagonal
    arrangement of the scaled learned queries -> one (128->32) matmul.
  - The global max subtractions inside the two exp() calls cancel exactly
    between numerator and denominator, so they are dropped.
  - The 1024-channel depthwise conv never materializes.  With
    r = 1/sum_den (computed as exp(-ln(den)), same ACT table set) define
        gamma[t,h,ij] = sum_q kern[t,q*8+h] * r[q*8+h,ij] * cost[n_t(ij),q*8+h]
    Then out_pre[(h,c),ij] = sum_t gamma[t,h,ij] * v[n_t(ij),(h,c)]  (256 ch)
    and out = Wout @ out_pre.
  - gamma's q-contraction + broadcast over the 32 channels of each head is a
    single small PE matmul per (tap, channel-chunk) with a one-hot*kern
    stationary operand; the tap accumulation is PSUM accumulation through
    identity matmuls.

Sharding: pure data parallel over batch: 16 batches -> 8 cores x 2.
"""

import os
from contextlib import ExitStack

import numpy as np

import concourse.bass as bass
import concourse.mybir as mybir
import concourse.tile as tile
from concourse import bacc
from concourse.bass_utils import run_bass_kernel_spmd

# Problem constants (hardcoded per spec nn_FusedKQnA_1726576854813)
N_Q, N_HEADS, KSIZE, STRIDE, PADDING = 4, 4, 3, 2, 1
B, C, H, W = 16, 128, 56, 56
HC = C // N_HEADS            # 32 head channels
HP = N_HEADS * STRIDE        # 8 effective heads
CS = C * STRIDE              # 256
G = N_Q * HP                 # 32 kernel groups
HO, WO = H // STRIDE, W // STRIDE   # 28, 28
NCORES = 8
BPC = B // NCORES            # batches per core

TAPS = [(di, dj) for di in (-1, 0, 1) for dj in (-1, 0, 1)]
N_STRIPS = 2                 # output rows split into strips of 14 (392 px)
ROWS_PER_STRIP = HO // N_STRIPS

F32 = mybir.dt.float32
BF16 = mybir.dt.bfloat16

_BUILD_CACHE = {}


def _host_weights(Wk, Wv, Wout, q_param, attn_scale, rpb_table):
    """Precompute all small weight tensors on the host."""
    q = q_param.reshape(N_Q, HP, HC).astype(np.float64) * (HC ** -0.5)
    QW = np.zeros((CS, G), np.float64)
    for qi in range(N_Q):
        for h in range(HP):
            QW[h * HC:(h + 1) * HC, qi * HP + h] = q[qi, h]
    wkq = (Wk.astype(np.float64) @ QW).astype(np.float32)        # (128, 32)

    rpb_exp = np.exp(rpb_table.astype(np.float64))               # (9, 32)
    kern_num = (rpb_exp * attn_scale.astype(np.float64))         # (9, 32)

    # denominator conv kernels as diagonal matmul weights: (9, G, G)
    denk = np.zeros((KSIZE * KSIZE, G, G), np.float32)
    for t in range(KSIZE * KSIZE):
        np.fill_diagonal(denk[t], rpb_exp[t])

    # gamma-broadcast stationary operands, stacked 3 taps per row-group for
    # tile_position packing: kmat[grp, ch][tau*32+g, m]
    kmat = np.zeros((3, 2, 3 * G, 128), np.float32)
    for t in range(KSIZE * KSIZE):
        grp, tau = divmod(t, 3)
        for ch in range(2):
            for g in range(G):
                h = g % HP
                if h // 4 == ch:
                    m0 = (h % 4) * HC
                    kmat[grp, ch, tau * G + g, m0:m0 + HC] = kern_num[t, g]

    woutT = np.ascontiguousarray(Wout.T.astype(np.float32))      # (256, 256) lhsT
    ident = np.eye(128, dtype=np.float32)
    import ml_dtypes
    return dict(wkq=wkq.astype(ml_dtypes.bfloat16), denk=denk, kmat=kmat,
                woutT=woutT, ident=ident,
                wv=np.ascontiguousarray(Wv.astype(ml_dtypes.bfloat16)))


def _build_program():
    """Build the Bass/Tile program once. Returns (nc, input_names)."""
    nc = bacc.Bacc("TRN2", target_bir_lowering=False, debug=False,
                   enable_asserts=False, num_devices=NCORES)

    x_d = nc.dram_tensor("x", [BPC, C, H, W], BF16, kind="ExternalInput").ap()
    wkq_d = nc.dram_tensor("wkq", [C, G], BF16, kind="ExternalInput").ap()
    wv_d = nc.dram_tensor("wv", [C, CS], BF16, kind="ExternalInput").ap()
    denk_d = nc.dram_tensor("denk", [9, G, G], F32, kind="ExternalInput").ap()
    kmat_d = nc.dram_tensor("kmat", [3, 2, 3 * G, 128], F32, kind="ExternalInput").ap()
    woutT_d = nc.dram_tensor("woutT", [CS, CS], F32, kind="ExternalInput").ap()
    ident_d = nc.dram_tensor("ident", [128, 128], F32, kind="ExternalInput").ap()
    out_d = nc.dram_tensor("out", [BPC, CS, HO, WO], F32, kind="ExternalOutput").ap()

    with tile.TileContext(nc) as tc, ExitStack() as ctx:
        _kernel_body(ctx, tc, out_d, x_d, wkq_d, wv_d, denk_d, kmat_d,
                     woutT_d, ident_d)

    _pin_act_tables()
    nc.compile()
    return nc


def _pin_act_tables():
    """Force one ACT table set (natural_log_exp_and_others) for Exp+Ln so the
    scheduler doesn't thrash table loads between them."""
    from concourse import hw_specs
    import concourse.bacc as bacc_mod
    if getattr(bacc_mod, "_act_tables_pinned", False):
        return
    orig = hw_specs.get_activation_tables

    def patched(arch):
        tabs = dict(orig(arch))
        keep = "natural_log_exp_and_others"
        for name in list(tabs):
            if name == keep:
                continue
            fns = tabs[name]
            if any(str(f).endswith((".Exp", ".Ln")) for f in fns):
                tabs[name] = type(fns)()
        return tabs

    bacc_mod.get_activation_tables = patched
    bacc_mod._act_tables_pinned = True


def _kernel_body(ctx, tc, out_d, x_d, wkq_d, wv_d, denk_d, kmat_d,
                 woutT_d, ident_d):
    nc = tc.nc
    PH, PW = H + 2, W + 2        # padded plane 58 x 58

    consts = ctx.enter_context(tc.tile_pool(name="consts", bufs=1))
    planes = ctx.enter_context(tc.tile_pool(name="planes", bufs=1))
    xpool = ctx.enter_context(tc.tile_pool(name="xpool", bufs=2))
    small = ctx.enter_context(tc.tile_pool(name="small", bufs=2))
    rcpool = ctx.enter_context(tc.tile_pool(name="rcpool", bufs=2))
    prod_pool = ctx.enter_context(tc.tile_pool(name="prod", bufs=4))
    # p tiles: 9 bufs so a full 3-group pipeline of products stays in flight
    opre_pool = ctx.enter_context(tc.tile_pool(name="opre", bufs=2))
    outs_pool = ctx.enter_context(tc.tile_pool(name="outs", bufs=4))

    ps = ctx.enter_context(tc.tile_pool(name="ps", bufs=2, space="PSUM"))

    # ---- constants into SBUF ----
    wkq_sb = consts.tile([C, G], BF16)
    nc.sync.dma_start(out=wkq_sb, in_=wkq_d)
    wv_sb = consts.tile([C, CS], BF16)
    nc.sync.dma_start(out=wv_sb, in_=wv_d)
    denk_sb = consts.tile([G, 9, G], BF16)
    nc.gpsimd.dma_start(out=denk_sb, in_=denk_d.transpose([1, 0, 2]))
    kmat_sb = consts.tile([3 * G, 3, 2, 128], BF16)
    nc.gpsimd.dma_start(out=kmat_sb, in_=kmat_d.transpose([2, 0, 1, 3]))

    woutT_sb = consts.tile([128, 2, CS], BF16)
    nc.gpsimd.dma_start(out=woutT_sb,
                        in_=woutT_d.rearrange("(kc k) m -> k kc m", kc=2))

    # ---- persistent padded planes (zero borders set once) ----
    cost_pl = [planes.tile([G, PH, PW], BF16, tag=f"cost{b}", name=f"cost_pl{b}") for b in range(BPC)]
    v_pl = [[planes.tile([128, PH, PW], BF16, tag=f"v{b}_{chn}", name=f"v_pl{b}_{chn}") for chn in range(2)]
            for b in range(BPC)]
    for pl in [cost_pl[b] for b in range(BPC)] + [v_pl[b][c] for b in range(BPC)
                                                 for c in range(2)]:
        # interior is fully overwritten every batch; only borders must be 0
        nc.gpsimd.memset(pl[:, 0, :], 0.0)
        nc.gpsimd.memset(pl[:, PH - 1, :], 0.0)
        nc.gpsimd.memset(pl[:, 1:PH - 1, 0], 0.0)
        nc.gpsimd.memset(pl[:, 1:PH - 1, PW - 1], 0.0)

    n_row_tiles = 7          # 56 rows in tiles of 8 -> matmul N=448
    RT = H // n_row_tiles    # 8 rows per tile

    def cview(b, di, dj):
        return cost_pl[b][:, 1 + di:1 + di + 2 * HO:2, 1 + dj:1 + dj + 2 * WO:2]

    # ---- phase A1: load x, qkT + exp (both batches) ----
    x_sb = {}
    for b in range(BPC):
        x_sb[b] = xpool.tile([C, H, W], BF16, name=f"x_sb{b}")
        nc.sync.dma_start(out=x_sb[b], in_=x_d[b])
    for b in range(BPC):
        for rt in range(n_row_tiles):
            qk_ps = ps.tile([G, RT, W], F32, tag="mm", bufs=2, name="qk_ps")
            nc.tensor.matmul(qk_ps, wkq_sb, x_sb[b][:, rt * RT:(rt + 1) * RT, :],
                             start=True, stop=True)
            nc.scalar.activation(
                out=cost_pl[b][:, 1 + rt * RT:1 + (rt + 1) * RT, 1:1 + W],
                in_=qk_ps, func=mybir.ActivationFunctionType.Exp)

    # ---- phase B: den conv as diagonal matmuls on PE + r = exp(-ln(den)) ----
    r_sb = {}
    for b in range(BPC):
        den_ps = ps.tile([G, 2, 512], F32, tag="gam", bufs=3, name="den_ps")
        for s in range(N_STRIPS):
            r0 = s * ROWS_PER_STRIP
            dv = den_ps[:, s, :ROWS_PER_STRIP * WO].rearrange(
                "p (a c) -> p a c", a=ROWS_PER_STRIP)
            for t, (di, dj) in enumerate(TAPS):
                nc.tensor.matmul(
                    dv, denk_sb[:, t, :],
                    cview(b, di, dj)[:, r0:r0 + ROWS_PER_STRIP, :],
                    start=(t == 0), stop=(t == 8))
        lden = small.tile([G, 2, ROWS_PER_STRIP, WO], F32, tag="lden",
                          name="lden")
        dfull = den_ps[:, :, :ROWS_PER_STRIP * WO].rearrange(
            "p s (a c) -> p s a c", a=ROWS_PER_STRIP)
        nc.scalar.activation(out=lden, in_=dfull,
                             func=mybir.ActivationFunctionType.Ln)
        r_sb[b] = small.tile([G, 2, ROWS_PER_STRIP, WO], BF16, tag="rr",
                             name="rr")
        nc.scalar.activation(out=r_sb[b], in_=lden, scale=-1.0,
                             func=mybir.ActivationFunctionType.Exp)

    # ---- phase A2: v matmuls -> v planes (both batches) ----
    for b in range(BPC):
        for chn in range(2):
            for rt in range(n_row_tiles):
                v_ps = ps.tile([128, RT, W], F32, tag="mm", bufs=2, name="v_ps")
                nc.tensor.matmul(v_ps, wv_sb[:, chn * 128:(chn + 1) * 128],
                                 x_sb[b][:, rt * RT:(rt + 1) * RT, :],
                                 start=True, stop=True)
                nc.scalar.copy(
                    out=v_pl[b][chn][:, 1 + rt * RT:1 + (rt + 1) * RT,
                                     1:1 + W],
                    in_=v_ps)

    # ---- phase C: rc[t] = cost_t * r (on GpSimd; stacked 3 taps/tile) ----
    rc_st = {}
    for b in range(BPC):
        rc_st[b] = [rcpool.tile([3 * G, 2, ROWS_PER_STRIP, WO], BF16,
                                tag=f"rc{grp}", name=f"rc_st{grp}")
                    for grp in range(3)]
        for t, (di, dj) in enumerate(TAPS):
            grp, tau = divmod(t, 3)
            cvs = cview(b, di, dj).rearrange("p (s a) c -> p s a c", s=2)
            eng = nc.gpsimd if t % 3 == 0 else nc.vector
            eng.tensor_mul(rc_st[b][grp][tau * G:(tau + 1) * G],
                           cvs, r_sb[b])

    # ---- phases D+E per batch: packed gamma matmuls, products, DVE
    #      bf16 tree accumulation over taps ----
    opre_sb = {}
    gsums = {}
    for grp in range(3):
        for b in range(BPC):
            for chn in range(2):
                gams = [ps.tile([128, 2, 512], F32, tag="gam", bufs=3,
                                name=f"gam_ps{tau}") for tau in range(3)]
                for s in range(N_STRIPS):
                    for tau in range(3):
                        gv = gams[tau][:, s, :ROWS_PER_STRIP * WO].rearrange(
                            "p (a c) -> p a c", a=ROWS_PER_STRIP)
                        nc.tensor.matmul(
                            gv, kmat_sb[tau * G:(tau + 1) * G, grp, chn, :],
                            rc_st[b][grp][tau * G:(tau + 1) * G, s],
                            start=True, stop=True,
                            tile_position=(tau * G, 0))
                ps_taps = []
                for tau in range(3):
                    t = grp * 3 + tau
                    di, dj = TAPS[t]
                    p_sb = prod_pool.tile([128, 2, ROWS_PER_STRIP, WO], BF16,
                                          tag="p", bufs=9, name=f"p{tau}")
                    gfull = gams[tau][:, :, :ROWS_PER_STRIP * WO].rearrange(
                        "p s (a c) -> p s a c", a=ROWS_PER_STRIP)
                    vv = v_pl[b][chn][:, 1 + di:1 + di + 2 * HO:2,
                                      1 + dj:1 + dj + 2 * WO:2].rearrange(
                        "p (s a) c -> p s a c", s=2)
                    nc.vector.tensor_mul(p_sb, gfull, vv)
                    ps_taps.append(p_sb)
                gs = prod_pool.tile([128, 2 * ROWS_PER_STRIP * WO], BF16,
                                    tag=f"gs{grp}_{b}_{chn}", bufs=1,
                                    name=f"gs{grp}{b}{chn}")
                flat = [p.rearrange("p s a c -> p (s a c)") for p in ps_taps]
                nc.vector.tensor_add(gs, flat[0], flat[1])
                nc.vector.tensor_add(gs, gs, flat[2])
                gsums[(grp, b, chn)] = gs
    for b in range(BPC):
        for chn in range(2):
            o_sb = opre_pool.tile([128, 2, ROWS_PER_STRIP, WO], BF16,
                                  tag=f"opre{chn}", name=f"opre{chn}")
            of = o_sb.rearrange("p s a c -> p (s a c)")
            nc.vector.tensor_add(of, gsums[(0, b, chn)], gsums[(1, b, chn)])
            nc.vector.tensor_add(of, of, gsums[(2, b, chn)])
            for s in range(N_STRIPS):
                opre_sb[(b, chn, s)] = o_sb[:, s]

        for mo in range(2):
            for s in range(N_STRIPS):
                out_ps = ps.tile([128, ROWS_PER_STRIP, WO], F32, tag="mm",
                                 bufs=2, name="out_ps")
                for kc in range(2):
                    nc.tensor.matmul(out_ps,
                                     woutT_sb[:, kc, mo * 128:(mo + 1) * 128],
                                     opre_sb[(b, kc, s)],
                                     start=(kc == 0), stop=(kc == 1))
                o_final = outs_pool.tile([128, ROWS_PER_STRIP, WO], F32)
                nc.scalar.copy(out=o_final, in_=out_ps)
                nc.sync.dma_start(
                    out=out_d[b, mo * 128:(mo + 1) * 128,
                              s * ROWS_PER_STRIP:(s + 1) * ROWS_PER_STRIP, :],
                    in_=o_final)


def _install_ntff_shim():
    """bass_utils expects antenv.axon_hooks (absent in this checkout); shim it
    with the ctypes NTFF hook from trn_agent_boot so trace=True works."""
    import sys
    import types
    try:
        from antenv.axon_hooks import get_axon_ntff_profile_hook  # noqa: F401
        return
    except ImportError:
        pass
    try:
        from trn_agent_boot.trn_boot import _ntff_profile_via_ctypes
        hook = _ntff_profile_via_ctypes("/opt/axon/libaxon_pjrt.so")
    except Exception:
        hook = None
    mod = types.ModuleType("antenv.axon_hooks")
    mod._hook = hook
    mod.get_axon_ntff_profile_hook = lambda: mod._hook
    mod.set_axon_ntff_profile_hook = lambda h: setattr(mod, "_hook", h)
    sys.modules["antenv.axon_hooks"] = mod


def _get_program():
    if "nc" not in _BUILD_CACHE:
        _BUILD_CACHE["nc"] = _build_program()
    return _BUILD_CACHE["nc"]


def kernel(x, Wk, Wv, Wout, q_param, attn_scale, rpb_table):
    import ml_dtypes
    x = np.ascontiguousarray(np.asarray(x, dtype=np.float32)
                             .astype(ml_dtypes.bfloat16))
    wts = _host_weights(np.asarray(Wk), np.asarray(Wv), np.asarray(Wout),
                        np.asarray(q_param), np.asarray(attn_scale),
                        np.asarray(rpb_table))
    nc = _get_program()

    in_maps = []
    for c in range(NCORES):
        in_maps.append({
            "x": np.ascontiguousarray(x[c * BPC:(c + 1) * BPC]),
            "wkq": wts["wkq"], "wv": wts["wv"], "denk": wts["denk"],
            "kmat": wts["kmat"], "woutT": wts["woutT"], "ident": wts["ident"],
        })

    trace = bool(int(os.environ.get("KERNEL_TRACE", "0")))
    if trace:
        _install_ntff_shim()
    res = run_bass_kernel_spmd(nc, in_maps, core_ids=list(range(NCORES)),
                               trace=trace)
    _BUILD_CACHE["last_results"] = res

    out = np.empty((B, CS, HO, WO), np.float32)
    for c in range(NCORES):
        out[c * BPC:(c + 1) * BPC] = res.results[c]["out"]
    return out
